# revision 5
# baseline (speedup 1.0000x reference)
"""Multi-head attention (B=4, S=2048, E=1024, H=16, D=64) on 8 TRN2 NeuronCores.

Sharding: core c handles batch b = c//2 and heads [8*(c%2), 8*(c%2)+8) —
data parallel over batch, tensor parallel over heads. No collectives.

Per-core structure (engine-balanced; the ScalarE exp stream is the critical
path, so everything else is scheduled to hide under it):
  - activations & weights arrive in bf16 (host converts; halves upload + HBM
    traffic); xq/xk resident in SBUF, xv streamed per 512-column chunk.
  - qT/kT projections per feature-chunk m (128 features = head pair (2m,2m+1))
    in bf16 x bf16 -> fp32 PSUM -> f32r SBUF.
  - v projection in N=256 groups (head pairs 2g,2g+1), fp32 PSUM -> fp16 vaug
    with a ones-column for the softmax denominator.
  - attention per head-pair m and sq-chunk j: for each sk block t, two K=64
    score matmuls packed onto the two PE row-tiles (tile_position
    (0,0)/(64,0)) writing separate PSUM banks; one exp ACT over both heads'
    scores (1024 elem/lane) -> fp16 P; two context matmuls (lhsT=[v|1] fp16)
    accumulating [65, 512] fp32 per head.
  - next-pair projection chains are emitted SPREAD across the current pair's
    j iterations (2 qk chains + one v chunk per j) so the PE absorbs them in
    per-j slack under the ACT stream instead of stalling at pair boundaries.
  output per core: [8 heads, 65, 2048] fp32; host divides rows 0..63 by row
  64, transposes, scatters into [4, 2048, 1024].
"""

import numpy as np
from contextlib import ExitStack

import concourse.bass as bass
import concourse.tile as tile
from concourse import bacc
from concourse import mybir
from concourse.bass_utils import run_bass_kernel_spmd

F32 = mybir.dt.float32
F32R = mybir.dt.float32r
F16 = mybir.dt.float16
BF16 = mybir.dt.bfloat16
EXP = mybir.ActivationFunctionType.Exp

B, S, E = 4, 2048, 1024
H, D = 16, 64
HPC = 8            # heads per core
FPC = HPC * D      # 512 output features per core
N_CORES = 8
KC = E // 128      # contraction chunks
NJ = S // 512      # sq chunks
NT = S // 128      # sk blocks
NM = FPC // 128    # feature chunks (head pairs)
SCALE = 0.125      # 1/sqrt(64)


def build_bass():
    nc = bacc.Bacc()
    xq = nc.declare_dram_parameter("xq_t", [E, S], BF16, isOutput=False)
    xk = nc.declare_dram_parameter("xk_t", [E, S], BF16, isOutput=False)
    xv = nc.declare_dram_parameter("xv_t", [E, S], BF16, isOutput=False)
    wq = nc.declare_dram_parameter("wq", [E, FPC], BF16, isOutput=False)
    wk = nc.declare_dram_parameter("wk", [E, FPC], BF16, isOutput=False)
    wv = nc.declare_dram_parameter("wv", [E, FPC], BF16, isOutput=False)
    out = nc.declare_dram_parameter("out", [HPC, D + 1, S], F32, isOutput=True)

    with tile.TileContext(nc) as tc, ExitStack() as ctx:
        sb = ctx.enter_context(tc.tile_pool(name="sb", bufs=1))
        xs = ctx.enter_context(tc.tile_pool(name="xs", bufs=2))
        exp = ctx.enter_context(tc.tile_pool(name="exp", bufs=2))
        ps = ctx.enter_context(tc.tile_pool(name="ps", bufs=1, space="PSUM"))

        # warm the exp table set first (the first real exp otherwise pays the
        # ~2.7us ACT_TABLE_LOAD on the critical path)
        warm = sb.tile([128, 16], F32, name="warm", tag="warm")
        nc.vector.memset(warm, 0.0)
        nc.scalar.activation(warm, warm, EXP)

        # --- weights + resident activations (bf16) ---
        w_sb = {}
        for name, w in (("wq", wq), ("wk", wk), ("wv", wv)):
            t = sb.tile([128, KC, FPC], BF16, name=f"{name}_sb", tag=f"{name}_sb")
            nc.sync.dma_start(out=t, in_=w.rearrange("(kc p) f -> p kc f", p=128))
            w_sb[name] = t
        x_sb = {}
        for name, x in (("xq", xq), ("xk", xk)):
            t = sb.tile([128, KC, S], BF16, name=f"{name}_sb", tag=f"{name}_sb")
            for j in range(NJ):
                nc.sync.dma_start(
                    out=t[:, :, j * 512:(j + 1) * 512],
                    in_=x[:, j * 512:(j + 1) * 512].rearrange(
                        "(kc p) f -> p kc f", p=128),
                )
            x_sb[name] = t

        # --- persistent projection outputs ---
        qT = sb.tile([128, NM, S], BF16, name="qT", tag="qT")   # [f%128, m, s]
        kT = sb.tile([128, NM, S], BF16, name="kT", tag="kT")
        vaug = sb.tile([128, HPC, NT, D + 1], F16, name="vaug", tag="vaug")
        nc.vector.memset(vaug[:, :, :, D:D + 1], 1.0)

        def qk_chain(wname, xname, dst, m, j):
            # one 8-matmul projection chain: feature chunk m, s columns j*512+
            acc = ps.tile([128, 512], F32, name=f"p_{wname}_{m}_{j}",
                          tag="proj", bufs=2)
            for kc in range(KC):
                nc.tensor.matmul(
                    acc,
                    lhsT=w_sb[wname][:, kc, m * 128:(m + 1) * 128],
                    rhs=x_sb[xname][:, kc, j * 512:(j + 1) * 512],
                    start=(kc == 0), stop=(kc == KC - 1),
                )
            nc.vector.tensor_copy(out=dst[:, m, j * 512:(j + 1) * 512], in_=acc)

        def v_chunk(g, j):
            # v features [256g, 256g+256) for s columns j*512+ into vaug.
            xt = xs.tile([128, KC, 512], BF16, name=f"x_v_{g}_{j}", tag="xt")
            nc.sync.dma_start(
                out=xt,
                in_=xv[:, j * 512:(j + 1) * 512].rearrange(
                    "(kc p) f -> p kc f", p=128),
            )
            for jo in range(4):
                sc = j * 4 + jo
                acc = ps.tile([128, 256], F32, name=f"p_v_{g}_{sc}",
                              tag="proj", bufs=2)
                for kc in range(KC):
                    nc.tensor.matmul(
                        acc,
                        lhsT=xt[:, kc, jo * 128:(jo + 1) * 128],
                        rhs=w_sb["wv"][:, kc, g * 256:(g + 1) * 256],
                        start=(kc == 0), stop=(kc == KC - 1),
                    )
                for hh in range(4):  # 4 heads in this group
                    h = g * 4 + hh
                    nc.vector.tensor_copy(
                        out=vaug[:, h, sc, 0:D],
                        in_=acc[:, hh * D:(hh + 1) * D])

        def attn_j(m, j):
            # attention for heads (2m, 2m+1), sq chunk j.
            cacc = ps.tile([D + 1, 2, 512], F32, name=f"ctx_{m}_{j}",
                           tag="cacc", bufs=1)
            for t in range(NT):
                st = ps.tile([128, 2, 512], F32, name=f"st_{m}_{j}_{t}",
                             tag="st", bufs=2)
                for u in range(2):
                    po = u * 64
                    nc.tensor.matmul(
                        st[:, u, :],
                        lhsT=kT[po:po + 64, m, t * 128:(t + 1) * 128],
                        rhs=qT[po:po + 64, m, j * 512:(j + 1) * 512],
                        start=True, stop=True,
                        tile_position=(po, 0),
                    )
                ex = exp.tile([128, 2, 512], F16, name=f"ex_{m}_{j}_{t}",
                              tag="ex")
                nc.scalar.activation(ex, st, EXP, scale=SCALE)
                for u in range(2):
                    nc.tensor.matmul(
                        cacc[:, u, :],
                        lhsT=vaug[:, 2 * m + u, t, :],
                        rhs=ex[:, u, :],
                        start=(t == 0), stop=(t == NT - 1),
                    )
            csb = exp.tile([D + 1, 2, 512], F32, name=f"csb_{m}_{j}",
                           tag="csb")
            nc.vector.tensor_copy(out=csb, in_=cacc)
            for u in range(2):
                nc.gpsimd.dma_start(
                    out=out[2 * m + u, :, j * 512:(j + 1) * 512],
                    in_=csb[:, u, :])

        # --- schedule (emission order = scheduler priority) ---
        # pre-phase: projections needed by attn pair 0 (+ pair 1's v),
        # in the same chain order as the v1 schedule (wq then wk, then v)
        for wname, xname, dst in (("wq", "xq", qT), ("wk", "xk", kT)):
            for j in range(NJ):
                qk_chain(wname, xname, dst, 0, j)
        for j in range(NJ):
            v_chunk(0, j)
        # main: per pair m, attention j-loop with next-pair projection chains
        # spread across the j iterations (emitted after each attn_j so they
        # sit just below it in priority and fill PE slack under ACT)
        for m in range(NM):
            for j in range(NJ):
                attn_j(m, j)
                if m + 1 < NM:
                    qk_chain("wq", "xq", qT, m + 1, j)
                    qk_chain("wk", "xk", kT, m + 1, j)
                if m == 0:
                    v_chunk(1, j)

    nc.compile()
    nc.freeze()
    return nc


_NC_CACHE = None


def _get_nc():
    global _NC_CACHE
    if _NC_CACHE is None:
        _NC_CACHE = build_bass()
    return _NC_CACHE


def prep_in_maps(inputs):
    import ml_dtypes
    bf16 = ml_dtypes.bfloat16
    queries = np.asarray(inputs["queries"], dtype=np.float32)
    keys = np.asarray(inputs["keys"], dtype=np.float32)
    values = np.asarray(inputs["values"], dtype=np.float32)
    Wq = np.asarray(inputs["Wq"], dtype=np.float32)
    Wk = np.asarray(inputs["Wk"], dtype=np.float32)
    Wv = np.asarray(inputs["Wv"], dtype=np.float32)

    xq_t = [np.ascontiguousarray(queries[b].T).astype(bf16) for b in range(B)]
    xk_t = [np.ascontiguousarray(keys[b].T).astype(bf16) for b in range(B)]
    xv_t = [np.ascontiguousarray(values[b].T).astype(bf16) for b in range(B)]
    w_half = [
        (np.ascontiguousarray(Wq[:, g * FPC:(g + 1) * FPC]).astype(bf16),
         np.ascontiguousarray(Wk[:, g * FPC:(g + 1) * FPC]).astype(bf16),
         np.ascontiguousarray(Wv[:, g * FPC:(g + 1) * FPC]).astype(bf16))
        for g in range(2)
    ]

    in_maps = []
    for c in range(N_CORES):
        b, g = c // 2, c % 2
        in_maps.append({
            "xq_t": xq_t[b], "xk_t": xk_t[b], "xv_t": xv_t[b],
            "wq": w_half[g][0], "wk": w_half[g][1], "wv": w_half[g][2],
        })
    return in_maps


def kernel(queries, keys, values, Wq, Wk, Wv, **_):
    in_maps = prep_in_maps(dict(
        queries=queries, keys=keys, values=values, Wq=Wq, Wk=Wk, Wv=Wv))

    nc = _get_nc()
    res = run_bass_kernel_spmd(nc, in_maps, list(range(N_CORES)))

    full = np.empty((B, S, H * D), dtype=np.float32)
    for c in range(N_CORES):
        b, g = c // 2, c % 2
        o = res.results[c]["out"]                # [HPC, D+1, S]
        ctx = o[:, :D, :] / o[:, D:D + 1, :]     # [HPC, D, S]
        dst = full[b].reshape(S, H, D)
        dst[:, g * HPC:(g + 1) * HPC, :] = ctx.transpose(2, 0, 1)
    return full
